# revision 37
# baseline (speedup 1.0000x reference)
"""Multi-head attention (B=2, T=2048, D=1024, H=16) on 8 TRN2 cores.

Sharding: core c -> batch b=c//4, head-group g=c%4 (4 heads, 256 proj cols).
Each core computes its 4 heads' attention + the partial out-projection
(O_g @ Wo[rows of g]); host sums the 4 partials per batch and adds
bo_eff = bo + bv @ Wo (exact fold: attention rows sum to 1, so bv passes
through attention unchanged; bk is softmax-invariant and dropped).

Device pipeline, per tq-block j of 512 (phases interleaved so attention of
block j overlaps the q/k/v streaming of block j+1):
  proj:  Q^T/K^T [dh, T] fp32r tiles (2 x 128 partitions = 2 heads each) and
         V' tiles [128 t, 4*65] with a ones column per head (denominator
         trick: row 64 of the PV accumulator is the softmax denominator).
  attn:  per head hl: S^T = K^T_blk^T Q^T_blk, P = exp(S/8) * causal_mask,
         O'^T[65,512] += V'^T P. Normalize via reciprocal of row 64 +
         partition_broadcast (input must sit at partition 0, hence the DMA
         hop).
  oproj: out[128t, 1024] = O^T^T @ Wo_rows accumulated over both row-halves.

All DMAs are issued from the SP engine (HWDGE): SWDGE on Pool costs ~1us
fixed per DMA and was the original bottleneck.
"""

import numpy as np
from contextlib import ExitStack

import concourse.tile as tile
from concourse import bacc, mybir
from concourse.bass_utils import run_bass_kernel_spmd

F32 = mybir.dt.float32
F32R = mybir.dt.float32r
BF16 = mybir.dt.bfloat16
AF = mybir.ActivationFunctionType

B, T, D, H, DH = 2, 2048, 1024, 16, 64
N_CORES = 8
HPC = 4            # heads per core
CS = HPC * DH      # 256 projection cols per core
NJ = T // 512      # 4 tq blocks
ND = D // 128      # 8 contraction blocks
NT = T // 128      # 16 t blocks
SCALE = 1.0 / 8.0  # 1/sqrt(DH)

_CACHE = {}


def _build():
    nc = bacc.Bacc("TRN2", target_bir_lowering=False, debug=False,
                   num_devices=N_CORES)
    qt_ap = nc.dram_tensor("qT", [D, T], F32, kind="ExternalInput").ap()
    kt_ap = nc.dram_tensor("kT", [D, T], F32, kind="ExternalInput").ap()
    vt_ap = nc.dram_tensor("vT", [D, T], F32, kind="ExternalInput").ap()
    wq_ap = nc.dram_tensor("wq", [D, CS], F32, kind="ExternalInput").ap()
    wk_ap = nc.dram_tensor("wk", [D, CS], F32, kind="ExternalInput").ap()
    wv_ap = nc.dram_tensor("wv", [D, CS], F32, kind="ExternalInput").ap()
    wo_ap = nc.dram_tensor("wo", [CS, D], F32, kind="ExternalInput").ap()
    bq_ap = nc.dram_tensor("bq", [CS], F32, kind="ExternalInput").ap()
    out_ap = nc.dram_tensor("out", [T, D], BF16, kind="ExternalOutput").ap()

    with tile.TileContext(nc) as tc, ExitStack() as ctx, \
            nc.allow_low_precision(reason="fp32r attention pipeline"):
        per = ctx.enter_context(tc.tile_pool(name="per", bufs=1))
        stream = ctx.enter_context(tc.tile_pool(name="stream", bufs=1))
        work = ctx.enter_context(tc.tile_pool(name="work", bufs=1))
        # PSUM: 8 banks total = ps_a(2) + ps_s(2x2-bank pairs) + ps_o(2)
        ps_a = ctx.enter_context(tc.tile_pool(name="ps_a", bufs=2, space="PSUM"))
        ps_s = ctx.enter_context(tc.tile_pool(name="ps_s", bufs=2, space="PSUM"))
        ps_o = ctx.enter_context(tc.tile_pool(name="ps_o", bufs=2, space="PSUM"))

        # ---- weight / stream loads (bulk traffic on the ACT HWDGE queue,
        # small latency-critical DMAs stay on the SP queue) ----
        def stream_tiles():
            qs = stream.tile([128, ND * 512], F32R, name="qs")
            ks = stream.tile([128, ND * 512], F32R, name="ks", bufs=2)
            vs = stream.tile([128, ND * 512], F32R, name="vs", bufs=2)
            return qs, ks, vs

        def load_one(ap_, sb_, jj):
            nc.scalar.dma_start(
                sb_.rearrange("p (i t) -> p i t", i=ND),
                ap_.rearrange("(i p) t -> p i t", p=128)
                   [:, :, 512 * jj:512 * (jj + 1)].bitcast(F32R),
            )

        wq_sb = per.tile([128, ND * CS], F32R)
        wk_sb = per.tile([128, ND * CS], F32R)
        wv_sb = per.tile([128, ND * CS], F32R)
        cur = stream_tiles()
        bq_sb = [per.tile([128, 1], F32, name=f"bq{ct}") for ct in range(2)]

        # two-queue startup: halves arrive just before the PE needs them;
        # wk/ks swap queues vs wq/qs so ks isn't head-of-line blocked
        def w_half(q, w_ap, w_sb, h):
            q.dma_start(
                w_sb.rearrange("p (i c) -> p i c", i=ND)[:, 4 * h:4 * h + 4, :],
                w_ap.rearrange("(i p) c -> p i c", p=128)
                    [:, 4 * h:4 * h + 4, :].bitcast(F32R),
            )

        def s_half(q, s_ap, s_sb, h):
            q.dma_start(
                s_sb.rearrange("p (i t) -> p i t", i=ND)[:, 4 * h:4 * h + 4, :],
                s_ap.rearrange("(i p) t -> p i t", p=128)
                    [:, 4 * h:4 * h + 4, 0:512].bitcast(F32R),
            )

        w_half(nc.sync, wq_ap, wq_sb, 0)
        s_half(nc.scalar, qt_ap, cur[0], 0)
        w_half(nc.sync, wq_ap, wq_sb, 1)
        s_half(nc.scalar, qt_ap, cur[0], 1)
        s_half(nc.sync, kt_ap, cur[1], 0)
        w_half(nc.scalar, wk_ap, wk_sb, 0)
        for ct in range(2):
            nc.sync.dma_start(
                bq_sb[ct][:], bq_ap[128 * ct:128 * (ct + 1)].unsqueeze(1))
        s_half(nc.sync, kt_ap, cur[1], 1)
        w_half(nc.scalar, wk_ap, wk_sb, 1)
        w_half(nc.sync, wv_ap, wv_sb, 0)
        s_half(nc.scalar, vt_ap, cur[2], 0)
        w_half(nc.sync, wv_ap, wv_sb, 1)
        s_half(nc.scalar, vt_ap, cur[2], 1)
        wo_sb = [per.tile([128, D], F32R, name=f"wo{ct}") for ct in range(2)]

        def load_wo():
            for ct in range(2):
                nc.sync.dma_start(
                    wo_sb[ct][:], wo_ap[128 * ct:128 * (ct + 1), :].bitcast(F32R))
        ones_sb = per.tile([128, HPC], F32)
        nc.gpsimd.memset(ones_sb[:], 1.0)
        masks = []
        for r in range(4):
            m = per.tile([128, 512], BF16, name=f"mask{r}")
            nc.gpsimd.memset(m[:], 1.0)
            nc.gpsimd.affine_select(
                out=m[:], in_=m[:], compare_op=mybir.AluOpType.is_ge,
                fill=0.0, base=-128 * r, pattern=[[1, 512]], channel_multiplier=-1,
            )
            masks.append(m)

        qT_sb = [per.tile([128, T], F32R, name=f"qT{ct}") for ct in range(2)]
        kT_sb = [per.tile([128, T], F32R, name=f"kT{ct}") for ct in range(2)]
        oT_sb = [per.tile([128, T], F32R, name=f"oT{ct}") for ct in range(2)]
        vp_sb = [per.tile([128, HPC * 65], F32R, name=f"vp{tt}") for tt in range(NT)]
        for tt in range(NT):
            nc.scalar.activation(
                vp_sb[tt].rearrange("p (h x) -> p h x", h=HPC)[:, :, 64:65],
                ones_sb.rearrange("p (h x) -> p h x", x=1),
                AF.Copy,
            )

        def proj(j, cur, nxt):
            qs, ks, vs = cur
            # Q/K projections; each next-block stream DMA is issued right
            # after the phase that frees its (bufs=1) buffer
            for w_sb, src, dst, bias, n_ap, n_sb in (
                (wq_sb, qs, qT_sb, bq_sb, qt_ap, nxt and nxt[0]),
                (wk_sb, ks, kT_sb, None, kt_ap, nxt and nxt[1]),
            ):
                for ct in range(2):
                    ps = ps_a.tile([128, 512], F32, name="a_ps")
                    for i in range(ND):
                        nc.tensor.matmul(
                            ps[:],
                            w_sb[:, CS * i + 128 * ct:CS * i + 128 * ct + 128],
                            src[:, 512 * i:512 * (i + 1)],
                            start=(i == 0), stop=(i == ND - 1),
                        )
                    if bias is not None:
                        nc.vector.tensor_scalar_add(
                            dst[ct][:, 512 * j:512 * (j + 1)],
                            ps[:], bias[ct][:, 0:1],
                        )
                    else:
                        nc.vector.tensor_copy(
                            dst[ct][:, 512 * j:512 * (j + 1)], ps[:],
                        )
                if nxt is not None:
                    load_one(n_ap, n_sb, j + 1)

            # ---- V projection ----
            for u in range(4):
                tt = 4 * j + u
                ps = ps_a.tile([128, 512], F32, name="a_ps")
                for i in range(ND):
                    nc.tensor.matmul(
                        ps[:, 0:CS],
                        vs[:, 512 * i + 128 * u:512 * i + 128 * (u + 1)],
                        wv_sb[:, CS * i:CS * (i + 1)],
                        start=(i == 0), stop=(i == ND - 1),
                    )
                nc.vector.tensor_copy(
                    vp_sb[tt].rearrange("p (h x) -> p h x", h=HPC)[:, :, 0:64],
                    ps[:, 0:CS].rearrange("p (h x) -> p h x", h=HPC),
                )
            if nxt is not None:
                load_one(vt_ap, nxt[2], j + 1)

        def attention(j):
            for hl in (1, 0, 3, 2):
                ct, po = hl // 2, 64 * (hl % 2)
                n_i = 4 * j + 4
                o_ps = ps_o.tile([65, 512], F32, name="o_ps")

                def s_mm(dst, i, c0):
                    nc.tensor.matmul(
                        dst,
                        kT_sb[ct][po:po + 64, 128 * i:128 * (i + 1)],
                        qT_sb[ct][po:po + 64, 512 * j + c0:512 * (j + 1)],
                        start=True, stop=True, skip_group_check=True,
                    )

                def pv_mm(p_ap, i, c0):
                    nc.tensor.matmul(
                        o_ps[:, c0:512], vp_sb[i][:, 65 * hl:65 * hl + 65], p_ap,
                        start=(i == 0), stop=(i == n_i - 1), skip_group_check=True,
                    )

                # off-diagonal blocks in pairs: one fused [128,1024] exp each
                for u in range(2 * j):
                    i0, i1 = 2 * u, 2 * u + 1
                    s2 = ps_s.tile([128, 1024], F32, name="s_ps")
                    s_mm(s2[:, 0:512], i0, 0)
                    s_mm(s2[:, 512:1024], i1, 0)
                    p2 = work.tile([128, 1024], F32R, name="p_sb", bufs=2)
                    nc.scalar.activation(p2[:], s2[:], AF.Exp, scale=SCALE)
                    pv_mm(p2[:, 0:512], i0, 0)
                    pv_mm(p2[:, 512:1024], i1, 0)

                # diagonal blocks: skip fully-masked columns (width >= 256
                # to stay off the fp32r narrow-free penalty)
                for r in range(4):
                    i = 4 * j + r
                    h = r % 2
                    if h == 0:
                        s2 = ps_s.tile([128, 1024], F32, name="s_ps")
                        p2 = work.tile([128, 1024], F32R, name="p_sb", bufs=2)
                    c0 = min(128 * r, 256)
                    lo, hi = 512 * h + c0, 512 * (h + 1)
                    s_mm(s2[:, lo:hi], i, c0)
                    nc.scalar.activation(p2[:, lo:hi], s2[:, lo:hi],
                                         AF.Exp, scale=SCALE)
                    nc.vector.tensor_mul(p2[:, lo:hi], p2[:, lo:hi],
                                         masks[r][:, c0:512])
                    pv_mm(p2[:, lo:hi], i, c0)
                r0 = work.tile([128, 512], F32, name="r0", bufs=2)
                nc.vector.reciprocal(r0[0:1, :], o_ps[64:65, :])
                bcast = work.tile([128, 512], F32, name="bcast", bufs=2)
                nc.gpsimd.partition_broadcast(bcast[0:64, :], r0[0:1, :])
                if po == 0:
                    nc.vector.tensor_mul(
                        oT_sb[ct][0:64, 512 * j:512 * (j + 1)],
                        o_ps[0:64, :], bcast[0:64, :],
                    )
                else:
                    nc.vector.tensor_mul(
                        oT_sb[ct][64:128, 512 * j:512 * (j + 1)],
                        o_ps[0:64, :], bcast[0:64, :],
                    )

        def oproj(j):
            for u in range(4):
                tt = 4 * j + u
                od = work.tile([128, D], BF16, name="odrain", bufs=2)
                for do in range(2):
                    op_ps = ps_a.tile([128, 512], F32, name="a_ps")
                    for ct in range(2):
                        nc.tensor.matmul(
                            op_ps[:],
                            oT_sb[ct][:, 128 * tt:128 * (tt + 1)],
                            wo_sb[ct][:, 512 * do:512 * (do + 1)],
                            start=(ct == 0), stop=(ct == 1),
                        )
                    nc.vector.tensor_copy(od[:, 512 * do:512 * (do + 1)], op_ps[:])
                nc.sync.dma_start(out_ap[128 * tt:128 * (tt + 1), :], od[:])

        # ---- schedule: projections are DMA-paced (10us PE vs 17us stream per
        # block), so interleave the small early attention blocks to fill the
        # stalls, then finish with the PE-dense late attention + oproj
        nxt = stream_tiles()
        proj(0, cur, nxt)
        cur, nxt = nxt, stream_tiles()
        proj(1, cur, nxt)
        attention(0)
        cur, nxt = nxt, stream_tiles()
        proj(2, cur, nxt)
        attention(1)
        proj(3, nxt, None)
        load_wo()
        attention(2)
        oproj(0)
        attention(3)
        oproj(1)
        oproj(2)
        oproj(3)

    nc.compile()
    return nc


def _get_nc():
    if "nc" not in _CACHE:
        _CACHE["nc"] = _build()
    return _CACHE["nc"]


def kernel(**inputs):
    q = np.asarray(inputs["q"], np.float32)
    k = np.asarray(inputs["k"], np.float32)
    v = np.asarray(inputs["v"], np.float32)
    Wq = np.asarray(inputs["Wq"], np.float32)
    Wk = np.asarray(inputs["Wk"], np.float32)
    Wv = np.asarray(inputs["Wv"], np.float32)
    Wo = np.asarray(inputs["Wo"], np.float32)
    bq = np.asarray(inputs["bq"], np.float32)
    bv = np.asarray(inputs["bv"], np.float32)
    bo = np.asarray(inputs["bo"], np.float32)

    nc = _get_nc()
    qT = [np.ascontiguousarray(q[b].T) for b in range(B)]
    kT = [np.ascontiguousarray(k[b].T) for b in range(B)]
    vT = [np.ascontiguousarray(v[b].T) for b in range(B)]
    in_maps = []
    for c in range(N_CORES):
        b, g = c // 4, c % 4
        cs = CS * g
        in_maps.append({
            "qT": qT[b], "kT": kT[b], "vT": vT[b],
            "wq": np.ascontiguousarray(Wq[:, cs:cs + CS]),
            "wk": np.ascontiguousarray(Wk[:, cs:cs + CS]),
            "wv": np.ascontiguousarray(Wv[:, cs:cs + CS]),
            "wo": np.ascontiguousarray(Wo[cs:cs + CS, :]),
            "bq": np.ascontiguousarray(bq[cs:cs + CS]),
        })
    res = run_bass_kernel_spmd(nc, in_maps, list(range(N_CORES)))

    bo_eff = bo.astype(np.float64) + bv.astype(np.float64) @ Wo.astype(np.float64)
    out = np.empty((B, T, D), np.float32)
    for b in range(B):
        acc = np.zeros((T, D), np.float64)
        for g in range(HPC):
            acc += res.results[4 * b + g]["out"].astype(np.float64)
        out[b] = (acc + bo_eff).astype(np.float32)
    return out


# revision 42
# speedup vs baseline: 1.1930x; 1.1930x over previous
"""Multi-head attention (B=2, T=2048, D=1024, H=16) on 8 TRN2 cores.

Sharding: core c -> batch b=c//4, head-group g=c%4 (4 heads, 256 proj cols).
Each core computes its 4 heads' attention + the partial out-projection
(O_g @ Wo[rows of g]); host sums the 4 partials per batch and adds
bo_eff = bo + bv @ Wo (exact fold: attention rows sum to 1, so bv passes
through attention unchanged; bk is softmax-invariant and dropped).

Device pipeline, per tq-block j of 512 (phases interleaved so attention of
block j overlaps the q/k/v streaming of block j+1):
  proj:  Q^T/K^T [dh, T] fp32r tiles (2 x 128 partitions = 2 heads each) and
         V' tiles [128 t, 4*65] with a ones column per head (denominator
         trick: row 64 of the PV accumulator is the softmax denominator).
  attn:  per head hl: S^T = K^T_blk^T Q^T_blk, P = exp(S/8) * causal_mask,
         O'^T[65,512] += V'^T P. Normalize via reciprocal of row 64 +
         partition_broadcast (input must sit at partition 0, hence the DMA
         hop).
  oproj: out[128t, 1024] = O^T^T @ Wo_rows accumulated over both row-halves.

All DMAs are issued from the SP engine (HWDGE): SWDGE on Pool costs ~1us
fixed per DMA and was the original bottleneck.
"""

import numpy as np
import ml_dtypes
from contextlib import ExitStack

import concourse.tile as tile
from concourse import bacc, mybir
from concourse.bass_utils import run_bass_kernel_spmd

F32 = mybir.dt.float32
F32R = mybir.dt.float32r
BF16 = mybir.dt.bfloat16
AF = mybir.ActivationFunctionType

B, T, D, H, DH = 2, 2048, 1024, 16, 64
N_CORES = 8
HPC = 4            # heads per core
CS = HPC * DH      # 256 projection cols per core
NJ = T // 512      # 4 tq blocks
ND = D // 128      # 8 contraction blocks
NT = T // 128      # 16 t blocks
SCALE = 1.0 / 8.0  # 1/sqrt(DH)

_CACHE = {}


def _build():
    nc = bacc.Bacc("TRN2", target_bir_lowering=False, debug=False,
                   num_devices=N_CORES)
    qt_ap = nc.dram_tensor("qT", [D, T], BF16, kind="ExternalInput").ap()
    kt_ap = nc.dram_tensor("kT", [D, T], BF16, kind="ExternalInput").ap()
    vt_ap = nc.dram_tensor("vT", [D, T], BF16, kind="ExternalInput").ap()
    wq_ap = nc.dram_tensor("wq", [D, CS], BF16, kind="ExternalInput").ap()
    wk_ap = nc.dram_tensor("wk", [D, CS], BF16, kind="ExternalInput").ap()
    wv_ap = nc.dram_tensor("wv", [D, CS], BF16, kind="ExternalInput").ap()
    wo_ap = nc.dram_tensor("wo", [CS, D], F32, kind="ExternalInput").ap()
    bq_ap = nc.dram_tensor("bq", [CS], F32, kind="ExternalInput").ap()
    out_ap = nc.dram_tensor("out", [T, D], BF16, kind="ExternalOutput").ap()

    with tile.TileContext(nc) as tc, ExitStack() as ctx, \
            nc.allow_low_precision(reason="fp32r attention pipeline"):
        per = ctx.enter_context(tc.tile_pool(name="per", bufs=1))
        stream = ctx.enter_context(tc.tile_pool(name="stream", bufs=1))
        work = ctx.enter_context(tc.tile_pool(name="work", bufs=1))
        # PSUM: 8 banks total = ps_a(2) + ps_s(2x2-bank pairs) + ps_o(2)
        ps_a = ctx.enter_context(tc.tile_pool(name="ps_a", bufs=2, space="PSUM"))
        ps_s = ctx.enter_context(tc.tile_pool(name="ps_s", bufs=2, space="PSUM"))
        ps_o = ctx.enter_context(tc.tile_pool(name="ps_o", bufs=2, space="PSUM"))

        # ---- weight / stream loads (bulk traffic on the ACT HWDGE queue,
        # small latency-critical DMAs stay on the SP queue) ----
        def stream_tiles():
            qs = stream.tile([128, ND * 512], BF16, name="qs")
            ks = stream.tile([128, ND * 512], BF16, name="ks", bufs=2)
            vs = stream.tile([128, ND * 512], BF16, name="vs", bufs=2)
            return qs, ks, vs

        def load_one(ap_, sb_, jj):
            nc.scalar.dma_start(
                sb_.rearrange("p (i t) -> p i t", i=ND),
                ap_.rearrange("(i p) t -> p i t", p=128)
                   [:, :, 512 * jj:512 * (jj + 1)],
            )

        wq_sb = per.tile([128, ND * CS], BF16)
        wk_sb = per.tile([128, ND * CS], BF16)
        wv_sb = per.tile([128, ND * CS], BF16)
        cur = stream_tiles()
        bq_sb = [per.tile([128, 1], F32, name=f"bq{ct}") for ct in range(2)]

        # two-queue startup: halves arrive just before the PE needs them;
        # wk/ks swap queues vs wq/qs so ks isn't head-of-line blocked
        def w_half(q, w_ap, w_sb, h):
            q.dma_start(
                w_sb.rearrange("p (i c) -> p i c", i=ND)[:, 4 * h:4 * h + 4, :],
                w_ap.rearrange("(i p) c -> p i c", p=128)
                    [:, 4 * h:4 * h + 4, :],
            )

        def s_half(q, s_ap, s_sb, h):
            q.dma_start(
                s_sb.rearrange("p (i t) -> p i t", i=ND)[:, 4 * h:4 * h + 4, :],
                s_ap.rearrange("(i p) t -> p i t", p=128)
                    [:, 4 * h:4 * h + 4, 0:512],
            )

        w_half(nc.sync, wq_ap, wq_sb, 0)
        s_half(nc.scalar, qt_ap, cur[0], 0)
        w_half(nc.sync, wq_ap, wq_sb, 1)
        s_half(nc.scalar, qt_ap, cur[0], 1)
        s_half(nc.sync, kt_ap, cur[1], 0)
        w_half(nc.scalar, wk_ap, wk_sb, 0)
        for ct in range(2):
            nc.sync.dma_start(
                bq_sb[ct][:], bq_ap[128 * ct:128 * (ct + 1)].unsqueeze(1))
        s_half(nc.sync, kt_ap, cur[1], 1)
        w_half(nc.scalar, wk_ap, wk_sb, 1)
        w_half(nc.sync, wv_ap, wv_sb, 0)
        s_half(nc.scalar, vt_ap, cur[2], 0)
        w_half(nc.sync, wv_ap, wv_sb, 1)
        s_half(nc.scalar, vt_ap, cur[2], 1)
        wo_sb = [per.tile([128, D], F32R, name=f"wo{ct}") for ct in range(2)]

        def load_wo():
            for ct in range(2):
                nc.sync.dma_start(
                    wo_sb[ct][:], wo_ap[128 * ct:128 * (ct + 1), :].bitcast(F32R))
        ones_sb = per.tile([128, HPC], F32)
        nc.gpsimd.memset(ones_sb[:], 1.0)
        masks = []
        for r in range(4):
            m = per.tile([128, 512], BF16, name=f"mask{r}")
            nc.gpsimd.memset(m[:], 1.0)
            nc.gpsimd.affine_select(
                out=m[:], in_=m[:], compare_op=mybir.AluOpType.is_ge,
                fill=0.0, base=-128 * r, pattern=[[1, 512]], channel_multiplier=-1,
            )
            masks.append(m)

        qT_sb = [per.tile([128, T], F32R, name=f"qT{ct}") for ct in range(2)]
        kT_sb = [per.tile([128, T], F32R, name=f"kT{ct}") for ct in range(2)]
        oT_sb = [per.tile([128, T], F32R, name=f"oT{ct}") for ct in range(2)]
        vp_sb = [per.tile([128, HPC * 65], F32R, name=f"vp{tt}") for tt in range(NT)]
        for tt in range(NT):
            nc.scalar.activation(
                vp_sb[tt].rearrange("p (h x) -> p h x", h=HPC)[:, :, 64:65],
                ones_sb.rearrange("p (h x) -> p h x", x=1),
                AF.Copy,
            )

        def proj(j, cur, nxt):
            qs, ks, vs = cur
            # Q/K projections; each next-block stream DMA is issued right
            # after the phase that frees its (bufs=1) buffer
            for w_sb, src, dst, bias, n_ap, n_sb in (
                (wq_sb, qs, qT_sb, bq_sb, qt_ap, nxt and nxt[0]),
                (wk_sb, ks, kT_sb, None, kt_ap, nxt and nxt[1]),
            ):
                for ct in range(2):
                    ps = ps_a.tile([128, 512], F32, name="a_ps")
                    for i in range(ND):
                        nc.tensor.matmul(
                            ps[:],
                            w_sb[:, CS * i + 128 * ct:CS * i + 128 * ct + 128],
                            src[:, 512 * i:512 * (i + 1)],
                            start=(i == 0), stop=(i == ND - 1),
                        )
                    if bias is not None:
                        nc.vector.tensor_scalar_add(
                            dst[ct][:, 512 * j:512 * (j + 1)],
                            ps[:], bias[ct][:, 0:1],
                        )
                    else:
                        nc.vector.tensor_copy(
                            dst[ct][:, 512 * j:512 * (j + 1)], ps[:],
                        )
                if nxt is not None:
                    load_one(n_ap, n_sb, j + 1)

            # ---- V projection ----
            for u in range(4):
                tt = 4 * j + u
                ps = ps_a.tile([128, 512], F32, name="a_ps")
                for i in range(ND):
                    nc.tensor.matmul(
                        ps[:, 0:CS],
                        vs[:, 512 * i + 128 * u:512 * i + 128 * (u + 1)],
                        wv_sb[:, CS * i:CS * (i + 1)],
                        start=(i == 0), stop=(i == ND - 1),
                    )
                nc.vector.tensor_copy(
                    vp_sb[tt].rearrange("p (h x) -> p h x", h=HPC)[:, :, 0:64],
                    ps[:, 0:CS].rearrange("p (h x) -> p h x", h=HPC),
                )
            if nxt is not None:
                load_one(vt_ap, nxt[2], j + 1)

        def attention(j):
            for hl in (1, 0, 3, 2):
                ct, po = hl // 2, 64 * (hl % 2)
                n_i = 4 * j + 4
                o_ps = ps_o.tile([65, 512], F32, name="o_ps")

                def s_mm(dst, i, c0):
                    nc.tensor.matmul(
                        dst,
                        kT_sb[ct][po:po + 64, 128 * i:128 * (i + 1)],
                        qT_sb[ct][po:po + 64, 512 * j + c0:512 * (j + 1)],
                        start=True, stop=True, skip_group_check=True,
                    )

                def pv_mm(p_ap, i, c0):
                    nc.tensor.matmul(
                        o_ps[:, c0:512], vp_sb[i][:, 65 * hl:65 * hl + 65], p_ap,
                        start=(i == 0), stop=(i == n_i - 1), skip_group_check=True,
                    )

                # off-diagonal blocks in pairs: one fused [128,1024] exp each
                for u in range(2 * j):
                    i0, i1 = 2 * u, 2 * u + 1
                    s2 = ps_s.tile([128, 1024], F32, name="s_ps")
                    s_mm(s2[:, 0:512], i0, 0)
                    s_mm(s2[:, 512:1024], i1, 0)
                    p2 = work.tile([128, 1024], F32R, name="p_sb", bufs=2)
                    nc.scalar.activation(p2[:], s2[:], AF.Exp, scale=SCALE)
                    pv_mm(p2[:, 0:512], i0, 0)
                    pv_mm(p2[:, 512:1024], i1, 0)

                # diagonal blocks: skip fully-masked columns (width >= 256
                # to stay off the fp32r narrow-free penalty)
                for r in range(4):
                    i = 4 * j + r
                    h = r % 2
                    if h == 0:
                        s2 = ps_s.tile([128, 1024], F32, name="s_ps")
                        p2 = work.tile([128, 1024], F32R, name="p_sb", bufs=2)
                    c0 = min(128 * r, 256)
                    lo, hi = 512 * h + c0, 512 * (h + 1)
                    s_mm(s2[:, lo:hi], i, c0)
                    nc.scalar.activation(p2[:, lo:hi], s2[:, lo:hi],
                                         AF.Exp, scale=SCALE)
                    nc.vector.tensor_mul(p2[:, lo:hi], p2[:, lo:hi],
                                         masks[r][:, c0:512])
                    pv_mm(p2[:, lo:hi], i, c0)
                r0 = work.tile([128, 512], F32, name="r0", bufs=2)
                nc.vector.reciprocal(r0[0:1, :], o_ps[64:65, :])
                bcast = work.tile([128, 512], F32, name="bcast", bufs=2)
                nc.gpsimd.partition_broadcast(bcast[0:64, :], r0[0:1, :])
                if po == 0:
                    nc.vector.tensor_mul(
                        oT_sb[ct][0:64, 512 * j:512 * (j + 1)],
                        o_ps[0:64, :], bcast[0:64, :],
                    )
                else:
                    nc.vector.tensor_mul(
                        oT_sb[ct][64:128, 512 * j:512 * (j + 1)],
                        o_ps[0:64, :], bcast[0:64, :],
                    )

        def oproj(j):
            for u in range(4):
                tt = 4 * j + u
                od = work.tile([128, D], BF16, name="odrain", bufs=2)
                for do in range(2):
                    op_ps = ps_a.tile([128, 512], F32, name="a_ps")
                    for ct in range(2):
                        nc.tensor.matmul(
                            op_ps[:],
                            oT_sb[ct][:, 128 * tt:128 * (tt + 1)],
                            wo_sb[ct][:, 512 * do:512 * (do + 1)],
                            start=(ct == 0), stop=(ct == 1),
                        )
                    nc.vector.tensor_copy(od[:, 512 * do:512 * (do + 1)], op_ps[:])
                nc.sync.dma_start(out_ap[128 * tt:128 * (tt + 1), :], od[:])

        # ---- schedule: projections are DMA-paced (10us PE vs 17us stream per
        # block), so interleave the small early attention blocks to fill the
        # stalls, then finish with the PE-dense late attention + oproj
        nxt = stream_tiles()
        proj(0, cur, nxt)
        cur, nxt = nxt, stream_tiles()
        proj(1, cur, nxt)
        attention(0)
        cur, nxt = nxt, stream_tiles()
        proj(2, cur, nxt)
        attention(1)
        proj(3, nxt, None)
        load_wo()
        attention(2)
        oproj(0)
        attention(3)
        oproj(1)
        oproj(2)
        oproj(3)

    nc.compile()
    return nc


def _get_nc():
    if "nc" not in _CACHE:
        _CACHE["nc"] = _build()
    return _CACHE["nc"]


def kernel(**inputs):
    q = np.asarray(inputs["q"], np.float32)
    k = np.asarray(inputs["k"], np.float32)
    v = np.asarray(inputs["v"], np.float32)
    Wq = np.asarray(inputs["Wq"], np.float32)
    Wk = np.asarray(inputs["Wk"], np.float32)
    Wv = np.asarray(inputs["Wv"], np.float32)
    Wo = np.asarray(inputs["Wo"], np.float32)
    bq = np.asarray(inputs["bq"], np.float32)
    bv = np.asarray(inputs["bv"], np.float32)
    bo = np.asarray(inputs["bo"], np.float32)

    nc = _get_nc()
    BF = ml_dtypes.bfloat16
    qT = [q[b].T.astype(BF) for b in range(B)]
    kT = [k[b].T.astype(BF) for b in range(B)]
    vT = [v[b].T.astype(BF) for b in range(B)]
    in_maps = []
    for c in range(N_CORES):
        b, g = c // 4, c % 4
        cs = CS * g
        in_maps.append({
            "qT": qT[b], "kT": kT[b], "vT": vT[b],
            "wq": Wq[:, cs:cs + CS].astype(BF),
            "wk": Wk[:, cs:cs + CS].astype(BF),
            "wv": Wv[:, cs:cs + CS].astype(BF),
            "wo": np.ascontiguousarray(Wo[cs:cs + CS, :]),
            "bq": np.ascontiguousarray(bq[cs:cs + CS]),
        })
    res = run_bass_kernel_spmd(nc, in_maps, list(range(N_CORES)))

    bo_eff = bo.astype(np.float64) + bv.astype(np.float64) @ Wo.astype(np.float64)
    out = np.empty((B, T, D), np.float32)
    for b in range(B):
        acc = np.zeros((T, D), np.float64)
        for g in range(HPC):
            acc += res.results[4 * b + g]["out"].astype(np.float64)
        out[b] = (acc + bo_eff).astype(np.float32)
    return out
